# revision 2
# baseline (speedup 1.0000x reference)
"""Grouped Conv2d (512 groups, 2->2 ch/group, 3x3 VALID) on 8 trn2 NeuronCores.

Strategy (hybrid, all-fp16 data path):
  - Shard the 512 groups across 8 cores: 64 groups = 128 channels per core.
    Fully independent (no collectives); batch stays whole on every core.
  - Row-split each batch's 54 output rows across engines:
      * PE rows [0, R_PE): block-diagonal 128x128 weight per 3x3 tap; 9
        accumulating fp16 matmuls per PSUM chunk (<=9 rows x 54 cols), ACT
        evicts PSUM->SBUF fp16.
      * DVE rows [R_PE, R_PE+R_DVE): per-term tensor_scalar product (4x
        mode) + tensor_tensor accumulate (2x mode), 18 terms = 9 taps x
        {diag, pair-swapped}.
      * MIX rows [R_PE+R_DVE, 54): ACT per-partition-scale products, Pool
        (gpsimd) tensor_tensor accumulates.
    Per-group 2x2 channel mixing needs x[p^1] at partition p: host supplies
    a channel pair-swapped copy `xs` of the bottom x rows.
  - fp16 halves DMA bytes vs fp32 (x on host; y converted back on host).
"""

import sys

import numpy as np

for _p in ("/opt/trn_rl_repo",):
    if _p not in sys.path:
        sys.path.insert(0, _p)

import concourse.bacc as bacc
import concourse.bass as bass
import concourse.tile as tile
from concourse import mybir
from concourse.bass_utils import run_bass_kernel_spmd

N_CORES = 8
B, C, H, W = 16, 1024, 56, 56
KH = KW = 3
HO, WO = H - KH + 1, W - KW + 1  # 54, 54
CPC = C // N_CORES  # 128 channels (64 groups) per core

R_PE = 43  # rows computed on the PE (psum chunks of <=9 rows)
R_DVE = 8  # rows computed on DVE
R_MIX = HO - R_PE - R_DVE  # rows computed by ACT products + Pool adds
VSTART = R_PE  # first vector-region row
XS_ROWS = R_DVE + R_MIX + KH - 1  # x rows needed by the vector region

# PSUM chunks for the PE region
_CHUNKS = []
_r = 0
while _r < R_PE:
    _CHUNKS.append((_r, min(9, R_PE - _r)))
    _r += min(9, R_PE - _r)

_NC_CACHE = {}


def _build_program():
    nc = bacc.Bacc(
        "TRN2", target_bir_lowering=False, debug=False, num_devices=N_CORES
    )
    f32 = mybir.dt.float32
    f16 = mybir.dt.float16

    x_d = nc.declare_dram_parameter("x", [B, CPC, H, W], f16, isOutput=False)
    xs_d = nc.declare_dram_parameter(
        "xs", [B, CPC, XS_ROWS, W], f16, isOutput=False
    )
    wm_d = nc.declare_dram_parameter(
        "wm", [CPC, KH * KW, CPC], f16, isOutput=False
    )
    wv_d = nc.declare_dram_parameter("wv", [CPC, 2, KH * KW], f32, isOutput=False)
    y_d = nc.declare_dram_parameter("y", [B, CPC, HO, WO], f16, isOutput=True)

    with tile.TileContext(nc) as tc:
        with (
            tc.tile_pool(name="wpool", bufs=1) as wpool,
            tc.tile_pool(name="xpool", bufs=3) as xpool,
            tc.tile_pool(name="xspool", bufs=3) as xspool,
            tc.tile_pool(name="opool", bufs=3) as opool,
            tc.tile_pool(name="tdpool", bufs=4) as tdpool,
            tc.tile_pool(name="tmpool", bufs=4) as tmpool,
            tc.tile_pool(name="psum", bufs=7, space="PSUM") as ppool,
            tc.tile_pool(name="scratch", bufs=1, space="PSUM") as spool,
        ):
            wt = wpool.tile([CPC, KH * KW, CPC], f16)
            nc.sync.dma_start(out=wt[:], in_=wm_d[:])
            wvt = wpool.tile([CPC, 2, KH * KW], f32)
            nc.sync.dma_start(out=wvt[:], in_=wv_d[:])

            # The fused matmul (LDW+MM) supports only ONE semaphore wait;
            # these sync matmuls absorb DMA waits so real matmuls only
            # depend on PE program order.
            scr = spool.tile([CPC, 512], f32)
            nc.tensor.matmul(
                scr[:, :2], lhsT=wt[:, 0, :], rhs=wt[:, 0, :2],
                start=True, stop=True,
            )
            # Dummy matmuls keep PE busy during the initial x DMA fill so
            # the HAM clock gate ramps to 2.4 GHz before real work arrives.
            for _ in range(16):
                nc.tensor.matmul(
                    scr[:, :256], lhsT=wt[:, 0, :], rhs=wt[:, 0:2, :],
                    start=True, stop=True,
                )

            for n in range(B):
                _emit_batch(
                    nc, xpool, xspool, opool, tdpool, tmpool, ppool,
                    x_d, xs_d, y_d, wt, wvt, scr, n,
                )
    nc.compile()
    return nc


def _emit_batch(
    nc, xpool, xspool, opool, tdpool, tmpool, ppool,
    x_d, xs_d, y_d, wt, wvt, scr, n,
):
    f32 = mybir.dt.float32
    f16 = mybir.dt.float16
    HSPLIT = 30
    OSPLIT = 27
    Copy = mybir.ActivationFunctionType.Copy
    add = mybir.AluOpType.add
    mult = mybir.AluOpType.mult

    xt = xpool.tile([CPC, H, W], f16)
    nc.sync.dma_start(out=xt[:, :HSPLIT, :], in_=x_d[n, :, :HSPLIT, :])
    nc.sync.dma_start(out=xt[:, HSPLIT:, :], in_=x_d[n, :, HSPLIT:, :])
    xst = xspool.tile([CPC, XS_ROWS, W], f16)
    nc.sync.dma_start(out=xst[:], in_=xs_d[n])

    # absorb the two x-DMA semaphores ahead of the real matmuls
    nc.tensor.matmul(
        scr[:, :2], lhsT=wt[:, 0, :], rhs=xt[:, 0, :2], start=True, stop=True
    )
    nc.tensor.matmul(
        scr[:, :2], lhsT=wt[:, 0, :], rhs=xt[:, H - 1, :2],
        start=True, stop=True,
    )

    ot = opool.tile([CPC, HO, WO], f16)

    # ---- PE region: accumulate 9 taps per PSUM chunk, ACT evicts ----
    for ci, (r0, nr) in enumerate(_CHUNKS):
        pt = ppool.tile([CPC, nr, WO], f32)
        t = 0
        for kh in range(KH):
            for kw in range(KW):
                nc.tensor.matmul(
                    pt[:],
                    lhsT=wt[:, t, :],
                    rhs=xt[:, r0 + kh : r0 + kh + nr, kw : kw + WO],
                    start=(t == 0),
                    stop=(t == KH * KW - 1),
                )
                t += 1
        nc.scalar.activation(ot[:, r0 : r0 + nr, :], pt[:], Copy)
        if r0 + nr == OSPLIT:
            nc.sync.dma_start(
                out=y_d[n, :, :OSPLIT, :], in_=ot[:, :OSPLIT, :]
            )

    # ---- vector region: 18 terms (9 taps x {diag, cross}) ----
    dv0 = VSTART  # DVE rows [dv0, dv0+R_DVE)
    mx0 = VSTART + R_DVE  # MIX rows [mx0, mx0+R_MIX)
    first = True
    for kh in range(KH):
        for kw in range(KW):
            t = kh * KW + kw
            for j in range(2):  # 0=diag, 1=cross (pair-swapped x)
                if j == 0:
                    dsl = xt[:, dv0 + kh : dv0 + kh + R_DVE, kw : kw + WO]
                    msl = xt[:, mx0 + kh : mx0 + kh + R_MIX, kw : kw + WO]
                else:
                    dsl = xst[:, kh : kh + R_DVE, kw : kw + WO]
                    msl = xst[
                        :, R_DVE + kh : R_DVE + kh + R_MIX, kw : kw + WO
                    ]
                sc = wvt[:, j, t : t + 1]
                od = ot[:, dv0 : dv0 + R_DVE, :]
                om = ot[:, mx0 : mx0 + R_MIX, :]
                if first:
                    nc.vector.tensor_scalar(
                        out=od, in0=dsl, scalar1=sc, scalar2=None, op0=mult
                    )
                    nc.scalar.activation(om, msl, Copy, scale=sc)
                    first = False
                else:
                    td = tdpool.tile([CPC, R_DVE, WO], f16)
                    nc.vector.tensor_scalar(
                        out=td[:], in0=dsl, scalar1=sc, scalar2=None, op0=mult
                    )
                    nc.vector.tensor_tensor(out=od, in0=od, in1=td[:], op=add)
                    tm = tmpool.tile([CPC, R_MIX, WO], f16)
                    nc.scalar.activation(tm[:], msl, Copy, scale=sc)
                    nc.gpsimd.tensor_tensor(out=om, in0=om, in1=tm[:], op=add)

    nc.sync.dma_start(out=y_d[n, :, OSPLIT:, :], in_=ot[:, OSPLIT:, :])


def _get_nc():
    if "nc" not in _NC_CACHE:
        _NC_CACHE["nc"] = _build_program()
    return _NC_CACHE["nc"]


def _make_wmats(w):
    """Per-core lhsT weight mats, shape (128, 9, 128): wm[ic, t, oc]."""
    oc = np.arange(CPC)
    mats = []
    for cid in range(N_CORES):
        ws = np.asarray(w[cid * CPC : (cid + 1) * CPC], dtype=np.float32)
        wm = np.zeros((CPC, KH * KW, CPC), dtype=np.float32)
        for icg in range(2):
            ic = (oc // 2) * 2 + icg
            wm[ic, :, oc] = ws[oc, icg].reshape(CPC, KH * KW)
        mats.append(wm.astype(np.float16))
    return mats


def _make_wvecs(w):
    """Per-core diag/cross scalar tables, shape (128, 2, 9) fp32."""
    p = np.arange(CPC)
    vecs = []
    for cid in range(N_CORES):
        ws = np.asarray(w[cid * CPC : (cid + 1) * CPC], dtype=np.float32)
        wv = np.empty((CPC, 2, KH * KW), dtype=np.float32)
        wv[:, 0, :] = ws[p, p % 2].reshape(CPC, KH * KW)
        wv[:, 1, :] = ws[p, 1 - p % 2].reshape(CPC, KH * KW)
        vecs.append(wv)
    return vecs


def _run(x, w, trace=False, **kwargs):
    nc = _get_nc()
    x = np.asarray(x)
    perm = np.arange(CPC) ^ 1
    wmats = _make_wmats(w)
    wvecs = _make_wvecs(w)
    in_maps = []
    for cid in range(N_CORES):
        xc = np.ascontiguousarray(
            x[:, cid * CPC : (cid + 1) * CPC], dtype=np.float16
        )
        xsc = np.ascontiguousarray(xc[:, perm, VSTART : VSTART + XS_ROWS, :])
        in_maps.append({"x": xc, "xs": xsc, "wm": wmats[cid], "wv": wvecs[cid]})
    res = run_bass_kernel_spmd(
        nc, in_maps, list(range(N_CORES)), trace=trace, **kwargs
    )
    y = np.concatenate(
        [res.results[i]["y"].astype(np.float32) for i in range(N_CORES)],
        axis=1,
    )
    return y, res


def kernel(x, w):
    y, _ = _run(x, w, trace=False)
    return y
